# revision 17
# baseline (speedup 1.0000x reference)
"""MoE MLP (E=32 experts, top-2, D=H=1024) on 8 Trainium2 NeuronCores.

Strategy (expert parallel, per sharding hint):
  * Host computes the (tiny) gate: softmax(x @ Wg), top-2, renormalized
    weights, and dispatches tokens per expert into capacity-padded blocks,
    transposed to [D, tokens] (features on SBUF partitions, tokens on the
    matmul moving/free dimension). This is the sharding/all-to-all step.
  * Each of the 8 cores owns 4 experts (W1/W2/b1/b2 shards) and computes
    GELU(x W1 + b1) W2 + b2 for its experts' token blocks.
  * Host combines with the top-2 gate weights (scatter-add).

Device kernel notes:
  * Weights are host-pre-tiled to [e, col_tile, partition, k_tile, 128] so
    each half-layer streams in as one fully-contiguous DMA chunk.
  * dma_start triggers cost ~0.4-0.8us serialized on the issuing engine's
    sequencer, so transfers are few and large, and triggers are spread
    across engine queues (sync=weights, vector=x, scalar=y, gpsimd=bias).
  * A short chain of dummy matmuls at kernel start warms the PE clock
    (HAM) while the first weight DMAs land.
"""

import os
import sys
import numpy as np

for _p in ("/root/.axon_site/_ro/trn_rl_repo", "/opt/trn_rl_repo"):
    if _p not in sys.path and os.path.isdir(_p):
        sys.path.append(_p)

E, D, H = 32, 1024, 1024
TOP_K = 2
N_CORES = 8
EPC = E // N_CORES  # experts per core
ND = D // 128       # d 128-tiles
NH = H // 128       # h 128-tiles

# weight dtype, activation dtype (must both be 16-bit or both 32-bit)
DT_W = os.environ.get("MOE_DT_W", "bfloat16")
DT_A = os.environ.get("MOE_DT_A", "bfloat16")
N_WARMUP_MM = int(os.environ.get("MOE_WARMUP", "20"))

LAST_EXEC_TIME_NS = None

_NC_CACHE = {}


def _build_nc(TCH, CW, dt_w_name, dt_a_name):
    import concourse.bass as bass  # noqa: F401
    import concourse.tile as tile
    from concourse import bacc, mybir
    from contextlib import ExitStack

    f32 = mybir.dt.float32
    dt_w = getattr(mybir.dt, dt_w_name)
    dt_a = getattr(mybir.dt, dt_a_name)
    C = TCH * CW

    nc = bacc.Bacc(
        "TRN2",
        target_bir_lowering=False,
        debug=False,
        enable_asserts=False,
        num_devices=N_CORES,
    )
    xT = nc.dram_tensor("xT", [D, EPC * C], dt_a, kind="ExternalInput").ap()
    # host-pre-tiled: w1[e, ht, p(=d_in), dt, hi], w2[e, dt, p(=h_in), ht, di]
    w1 = nc.dram_tensor("w1", [EPC, NH, 128, ND, 128], dt_w, kind="ExternalInput").ap()
    w2 = nc.dram_tensor("w2", [EPC, ND, 128, NH, 128], dt_w, kind="ExternalInput").ap()
    # host-pre-transposed biases: [p, e, col_tile]
    b1 = nc.dram_tensor("b1", [128, EPC, NH], f32, kind="ExternalInput").ap()
    b2 = nc.dram_tensor("b2", [128, EPC, ND], f32, kind="ExternalInput").ap()
    yT = nc.dram_tensor("yT", [D, EPC * C], f32, kind="ExternalOutput").ap()

    HNH = NH // 2  # half-layer column split
    HND = ND // 2
    # weight-pool lookahead: 4-byte weights are SBUF-tight
    WB = 3 if mybir.dt.size(dt_w) == 4 else 4

    with tile.TileContext(nc) as tc, ExitStack() as ctx:
        wpool = ctx.enter_context(tc.tile_pool(name="w", bufs=4))
        xpool = ctx.enter_context(tc.tile_pool(name="x", bufs=2))
        hpool = ctx.enter_context(tc.tile_pool(name="h", bufs=2 * NH))
        ypool = ctx.enter_context(tc.tile_pool(name="y", bufs=2))
        bpool = ctx.enter_context(tc.tile_pool(name="b", bufs=1))
        pp1 = ctx.enter_context(tc.tile_pool(name="ps1", bufs=3, space="PSUM"))
        pp2 = ctx.enter_context(tc.tile_pool(name="ps2", bufs=3, space="PSUM"))
        ppw = ctx.enter_context(tc.tile_pool(name="psw", bufs=1, space="PSUM"))

        # PE warm-up: dummy matmuls with no DMA dependency keep the PE
        # busy from t~0 so HAM un-throttles before the real matmuls.
        if N_WARMUP_MM:
            wu = bpool.tile([128, 512], mybir.dt.bfloat16, tag="wu")
            nc.vector.memset(wu[:], 0.0)
            wups = ppw.tile([128, 512], f32, tag="psw")
            for i in range(N_WARMUP_MM):
                nc.tensor.matmul(wups[:], wu[:, :128], wu[:],
                                 start=(i == 0), stop=(i == N_WARMUP_MM - 1))

        gelu = mybir.ActivationFunctionType.Gelu
        b1_sb = b2_sb = None
        for e in range(EPC):
            # tokens: one DMA per expert -> [p, (dt, tok)]
            xt = xpool.tile([128, ND * C], dt_a, tag="xt")
            nc.gpsimd.dma_start(
                out=xt[:].rearrange("p (dt t) -> p dt t", dt=ND),
                in_=xT[:, e * C:(e + 1) * C].rearrange("(dt p) t -> p dt t", p=128),
            )
            # weights: W1 in column chunks (quarters for the first expert so
            # compute starts on the first 512KB), in consumption order
            n_chunks = 4 if e == 0 else 2
            csz = NH // n_chunks
            w1h = []
            for half in range(n_chunks):
                wt = wpool.tile([128, csz * ND * 128], dt_w,
                                tag=f"w1c{n_chunks}",
                                bufs=(4 if n_chunks == 4 else WB))
                nc.sync.dma_start(
                    out=wt[:].rearrange("p (ht dt hi) -> p ht dt hi", ht=csz, dt=ND),
                    in_=w1[e, half * csz:(half + 1) * csz].rearrange(
                        "ht p dt hi -> p ht dt hi"),
                )
                w1h.append(wt)
            if b1_sb is None:
                b1_sb = bpool.tile([128, EPC * NH], f32, tag="b1")
                b2_sb = bpool.tile([128, EPC * ND], f32, tag="b2")
                nc.gpsimd.dma_start(
                    out=b1_sb[:].rearrange("p (e ht) -> p e ht", e=EPC), in_=b1[:])
                nc.gpsimd.dma_start(
                    out=b2_sb[:].rearrange("p (e dt) -> p e dt", e=EPC), in_=b2[:])
            w2h = []
            for half in range(2):
                wt = wpool.tile([128, HND * NH * 128], dt_w, tag="w2c",
                                bufs=WB)
                nc.sync.dma_start(
                    out=wt[:].rearrange("p (dt ht di) -> p dt ht di", dt=HND, ht=NH),
                    in_=w2[e, half * HND:(half + 1) * HND].rearrange(
                        "dt p ht di -> p dt ht di"),
                )
                w2h.append(wt)

            for ch in range(TCH):
                hts = []
                for ht in range(NH):
                    wt = w1h[ht // csz]
                    hoff = (ht % csz) * ND * 128
                    ps = pp1.tile([128, CW], f32, tag="ps1")
                    for dt_i in range(ND):
                        nc.tensor.matmul(
                            ps[:],
                            wt[:, hoff + dt_i * 128: hoff + (dt_i + 1) * 128],
                            xt[:, dt_i * C + ch * CW: dt_i * C + (ch + 1) * CW],
                            start=(dt_i == 0),
                            stop=(dt_i == ND - 1),
                        )
                    hsb = hpool.tile([128, CW], dt_a, tag="ht")
                    nc.scalar.activation(
                        hsb[:], ps[:], gelu,
                        bias=b1_sb[:, e * NH + ht: e * NH + ht + 1],
                    )
                    hts.append(hsb)
                ysb = ypool.tile([128, ND * CW], f32, tag="yt")
                for dt_i in range(ND):
                    wt = w2h[dt_i // HND]
                    doff = (dt_i % HND) * NH * 128
                    ps2 = pp2.tile([128, CW], f32, tag="ps2")
                    for ht in range(NH):
                        nc.tensor.matmul(
                            ps2[:],
                            wt[:, doff + ht * 128: doff + (ht + 1) * 128],
                            hts[ht][:],
                            start=(ht == 0),
                            stop=(ht == NH - 1),
                        )
                    nc.vector.tensor_scalar_add(
                        ysb[:, dt_i * CW:(dt_i + 1) * CW], ps2[:],
                        b2_sb[:, e * ND + dt_i: e * ND + dt_i + 1],
                    )
                for half in range(2):
                    r0, r1 = half * HND * 128, (half + 1) * HND * 128
                    nc.scalar.dma_start(
                        out=yT[r0:r1, e * C + ch * CW: e * C + (ch + 1) * CW]
                        .rearrange("(dt p) t -> p dt t", p=128),
                        in_=ysb[:, half * HND * CW:(half + 1) * HND * CW]
                        .rearrange("p (dt t) -> p dt t", dt=HND),
                    )
    nc.compile()
    return nc


def _get_nc(TCH, CW, dt_w, dt_a):
    key = (TCH, CW, dt_w, dt_a)
    if key not in _NC_CACHE:
        _NC_CACHE[key] = _build_nc(TCH, CW, dt_w, dt_a)
    return _NC_CACHE[key]


def _np_dt(name):
    if name == "bfloat16":
        import ml_dtypes
        return np.dtype(ml_dtypes.bfloat16)
    return np.dtype(np.float32)


def _route(xf, Wg):
    """Replicates the reference gate exactly in f32 numpy."""
    logits = xf @ Wg                                     # [T, E]
    m = logits.max(-1, keepdims=True)
    ex = np.exp(logits - m)
    scores = ex / ex.sum(-1, keepdims=True)
    idx = np.argsort(-scores, axis=1, kind="stable")[:, :TOP_K]  # [T, k]
    tw = np.take_along_axis(scores, idx, 1)
    m2 = tw.max(-1, keepdims=True)
    e2 = np.exp(tw - m2)
    w = (e2 / e2.sum(-1, keepdims=True)).astype(np.float32)
    return idx.astype(np.int64), w


def kernel(x, Wg, W1, b1, W2, b2):
    global LAST_EXEC_TIME_NS
    from concourse import bass_utils

    dt_w, dt_a = DT_W, DT_A
    orig_shape = x.shape
    x = np.asarray(x, dtype=np.float32)
    Wg = np.asarray(Wg, dtype=np.float32)
    W1 = np.asarray(W1, dtype=np.float32)
    b1 = np.asarray(b1, dtype=np.float32)
    W2 = np.asarray(W2, dtype=np.float32)
    b2 = np.asarray(b2, dtype=np.float32)
    xf = np.ascontiguousarray(x.reshape(-1, D))
    T = xf.shape[0]

    idx, w = _route(xf, Wg)

    # ---- dispatch: per-expert capacity-padded token blocks
    flat_e = idx.reshape(-1)                 # [k*T]
    flat_t = np.repeat(np.arange(T), TOP_K)
    order = np.argsort(flat_e, kind="stable")
    counts = np.bincount(flat_e, minlength=E)
    maxc = int(counts.max())
    C = max(256, -(-maxc // 16) * 16)
    TCH = -(-C // 512)
    CW = -(-C // (TCH * 16)) * 16
    C = TCH * CW

    starts = np.zeros(E + 1, np.int64)
    starts[1:] = np.cumsum(counts)
    se = flat_e[order]
    pos = np.arange(TOP_K * T) - starts[se]
    core = se // EPC
    col = (se % EPC) * C + pos               # column in that core's xT
    tok = flat_t[order]

    gidx = np.zeros((N_CORES, EPC * C), np.int64)
    for c in range(N_CORES):
        msel = core == c
        gidx[c, col[msel]] = tok[msel]

    np_w = _np_dt(dt_w)
    np_a = _np_dt(dt_a)
    xf_a = xf.astype(np_a, copy=False)
    # pre-tile weights: w1 -> [e, ht, p(d_in), dt, hi], w2 -> [e, dt, p(h_in), ht, di]
    W1t = np.ascontiguousarray(
        W1.reshape(E, ND, 128, NH, 128).transpose(0, 3, 2, 1, 4).astype(np_w, copy=False))
    W2t = np.ascontiguousarray(
        W2.reshape(E, NH, 128, ND, 128).transpose(0, 3, 2, 1, 4).astype(np_w, copy=False))
    # pre-transpose biases to [p, e, col_tile]
    b1t = np.ascontiguousarray(b1.reshape(E, NH, 128).transpose(2, 0, 1))
    b2t = np.ascontiguousarray(b2.reshape(E, ND, 128).transpose(2, 0, 1))

    in_maps = []
    for c in range(N_CORES):
        e0 = c * EPC
        in_maps.append({
            "xT": np.ascontiguousarray(xf_a[gidx[c]].T),
            "w1": W1t[e0:e0 + EPC],
            "w2": W2t[e0:e0 + EPC],
            "b1": b1t[:, e0:e0 + EPC],
            "b2": b2t[:, e0:e0 + EPC],
        })

    nc = _get_nc(TCH, CW, dt_w, dt_a)
    trace = os.environ.get("MOE_TRACE", "0") == "1"
    res = bass_utils.run_bass_kernel_spmd(
        nc, in_maps, core_ids=list(range(N_CORES)), trace=trace,
    )
    LAST_EXEC_TIME_NS = res.exec_time_ns

    # ---- combine: gather each (token, k) contribution, weight, and sum
    Ystack = np.stack([res.results[c]["yT"].T for c in range(N_CORES)])
    contrib = Ystack[core, col]              # [k*T, D] (sorted order)
    inv = np.empty_like(order)
    inv[order] = np.arange(TOP_K * T)
    contrib = contrib[inv].reshape(T, TOP_K, D)
    y = (contrib * w[:, :, None]).sum(1).astype(np.float32)
    return y.reshape(orig_shape)
